# revision 16
# baseline (speedup 1.0000x reference)
"""Causal attention head (RoPE) kernel for 8 Trainium2 NeuronCores.

Sharding: 8 cores = 2 batches x 4 head-groups (4 heads each), no
cross-device comms. v3 design:

  - Q/K projections as fp8 DoubleRow matmuls (K_eff=256 per instruction,
    ~1.3x faster than bf16 chains) in 512-col PSUM chunks with baseline
    weight layout (per head: 32 even-RoPE rows then 32 odd rows).
  - RoPE per chunk: a = ps*cos, ap = ps*sin' on DVE (bf16 out), partition
    swap of the sin product via 4 block DMAs (dispatched from the Pool
    sequencer, ~25ns each), sub on Pool -> qt/kt bf16 [128, T].
  - V projected straight into natural (key-major) layout with x^T bf16 as
    the stationary operand (no PE transposes); ones column per head so
    PV row 64 is the softmax denominator.
  - scores bf16 dual-tile (two heads on PE row-tiles 0-63/64-127, K=64),
    j-major over key blocks with 1024-wide segments; exp on ACT (~700ns
    fixed cost per instruction, so wide exps) writes bf16 P^T tiles.
  - PV issue lagged 2 key-blocks behind scores so the PE never waits on
    exp (p-state: any PE gap drops the clock 2.4->1.2GHz for 3us).
  - po [65, 512] PSUM -> DVE copy -> HBM unnormalized; the host divides
    by the denominator row on gather (free).
"""

import os
import sys
from contextlib import ExitStack

import numpy as np

for _p in ("/opt/trn_rl_repo", "/root/.axon_site/_ro/trn_rl_repo"):
    if os.path.isdir(_p) and _p not in sys.path:
        sys.path.append(_p)

import ml_dtypes

import concourse.bass as bass
import concourse.mybir as mybir
import concourse.tile as tile
from concourse import bacc
from concourse.bass_utils import run_bass_kernel_spmd

P = 128
T = 2048
CIN = 1024
NHC = 4          # heads per core
HS = 64
DOUT = NHC * HS  # 256
NCT = CIN // P   # 8 contraction tiles
NCP = NCT // 2   # 4 fp8 DoubleRow contraction pairs
SCALE = 1.0 / 32.0  # 1024 ** -0.5

F32 = mybir.dt.float32
BF16 = mybir.dt.bfloat16
F8 = mybir.dt.float8e4
DR = mybir.MatmulPerfMode.DoubleRow


def _build_nc():
    nc = bacc.Bacc("TRN2")

    x8T = nc.dram_tensor("x8T", [CIN, T], F8, kind="ExternalInput").ap()
    xbT = nc.dram_tensor("xbT", [CIN, T], BF16, kind="ExternalInput").ap()
    wq8 = nc.dram_tensor("wq8", [CIN, DOUT], F8, kind="ExternalInput").ap()
    wk8 = nc.dram_tensor("wk8", [CIN, DOUT], F8, kind="ExternalInput").ap()
    wvT = nc.dram_tensor("wvT", [CIN, DOUT], BF16, kind="ExternalInput").ap()
    cos4 = nc.dram_tensor("cos4", [P, T], BF16, kind="ExternalInput").ap()
    sin4 = nc.dram_tensor("sin4", [P, T], BF16, kind="ExternalInput").ap()
    utri = nc.dram_tensor("utri", [P, P], BF16, kind="ExternalInput").ap()
    outT = nc.dram_tensor("outT", [NHC * (HS + 1), T], F32, kind="ExternalOutput").ap()

    with tile.TileContext(nc) as tc, ExitStack() as ctx:
        const_pool = ctx.enter_context(tc.tile_pool(name="const", bufs=1))
        wpool = ctx.enter_context(tc.tile_pool(name="w", bufs=1))
        xpool = ctx.enter_context(tc.tile_pool(name="x", bufs=1))
        qkpool = ctx.enter_context(tc.tile_pool(name="qk", bufs=1))
        mpool = ctx.enter_context(tc.tile_pool(name="m", bufs=3))
        vpool = ctx.enter_context(tc.tile_pool(name="vaug", bufs=1))
        ptpool = ctx.enter_context(tc.tile_pool(name="pt", bufs=1))
        opool = ctx.enter_context(tc.tile_pool(name="ob", bufs=3))

        pp_acc = ctx.enter_context(tc.tile_pool(name="pp_acc", bufs=2, space="PSUM"))
        pp_s = ctx.enter_context(tc.tile_pool(name="pp_s", bufs=2, space="PSUM"))
        pp_po = ctx.enter_context(tc.tile_pool(name="pp_po", bufs=2, space="PSUM"))

        # ---- input DMAs: weights + fp8 x first (projections start on them),
        # bf16 x (V path) + trig on the scalar queue, small consts late.
        w_tiles = {}
        for name, wsrc, dt in (("q", wq8, F8), ("k", wk8, F8)):
            w_s = wpool.tile([P, NCT * DOUT], dt, tag=f"w{name}", name=f"w{name}")
            nc.sync.dma_start(
                w_s.rearrange("p (n d) -> p n d", n=NCT),
                wsrc.rearrange("(n p) d -> p n d", p=P),
            )
            w_tiles[name] = w_s
        xs8 = xpool.tile([P, NCT * T], F8, tag="xs8")
        x8_r = x8T.rearrange("(n p) t -> p n t", p=P)
        xs8_r = xs8.rearrange("p (n t) -> p n t", n=NCT)
        for ch in range(4):
            cs = slice(ch * 512, (ch + 1) * 512)
            nc.sync.dma_start(xs8_r[:, :, cs], x8_r[:, :, cs])
        wv_s = wpool.tile([P, NCT * DOUT], BF16, tag="wv", name="wv")
        nc.sync.dma_start(
            wv_s.rearrange("p (n d) -> p n d", n=NCT),
            wvT.rearrange("(n p) d -> p n d", p=P),
        )
        w_tiles["v"] = wv_s
        cos_s = const_pool.tile([P, T], BF16, tag="cos")
        nc.scalar.dma_start(cos_s[:], cos4)
        sin_s = const_pool.tile([P, T], BF16, tag="sin")
        nc.scalar.dma_start(sin_s[:], sin4)
        utri_s = const_pool.tile([P, P], BF16, tag="utri")
        nc.gpsimd.dma_start(utri_s[:], utri)
        xsb = xpool.tile([P, NCT * T], BF16, tag="xsb")
        xb_r = xbT.rearrange("(n p) t -> p n t", p=P)
        xsb_r = xsb.rearrange("p (n t) -> p n t", n=NCT)
        for ch in range(4):
            cs = slice(ch * 512, (ch + 1) * 512)
            nc.gpsimd.dma_start(xsb_r[:, :, cs], xb_r[:, :, cs])

        # persistent SBUF: roped q/k bf16 (head pair m: rows 0-63 / 64-127)
        qt = [qkpool.tile([P, T], BF16, tag=f"qt{m}", name=f"qt{m}") for m in range(2)]
        kt = [qkpool.tile([P, T], BF16, tag=f"kt{m}", name=f"kt{m}") for m in range(2)]
        va = [
            vpool.tile([P, NHC * (HS + 1)], BF16, tag=f"vaug{tb}", name=f"vaug{tb}")
            for tb in range(T // P)
        ]

        _swap_engs = (nc.gpsimd, nc.sync, nc.scalar, nc.gpsimd)

        def proj_rope_chunk(m, wname, dst, ch):
            """fp8 DR projection of one 512-col chunk of an m-tile + RoPE."""
            w_r = w_tiles[wname].rearrange("p (n d) -> p n d", n=NCT)
            cs = slice(ch * 512, (ch + 1) * 512)
            ps = pp_acc.tile([P, 512], F32, tag="acc", name=f"pj{wname}{m}{ch}")
            for cp in range(NCP):
                nc.tensor.matmul(
                    ps[:],
                    lhsT=w_r[:, 2 * cp:2 * cp + 2, m * P:(m + 1) * P],
                    rhs=xs8_r[:, 2 * cp:2 * cp + 2, cs],
                    perf_mode=DR,
                    start=(cp == 0),
                    stop=(cp == NCP - 1),
                )
            a = mpool.tile([P, 512], BF16, tag="ra")
            ap = mpool.tile([P, 512], BF16, tag="rp")
            nc.vector.tensor_mul(a[:], ps[:], cos_s[:, cs])
            nc.vector.tensor_mul(ap[:], ps[:], sin_s[:, cs])
            sw = mpool.tile([P, 512], BF16, tag="rs")
            for blk in range(4):
                s0 = (blk ^ 1) * 32
                _swap_engs[blk].dma_start(
                    sw[blk * 32:(blk + 1) * 32, :], ap[s0:s0 + 32, :]
                )
            nc.vector.tensor_sub(dst[:, cs], a[:], sw[:])

        def vproj(tbp):
            """bf16 V proj of t-blocks (2*tbp, 2*tbp+1) into natural layout."""
            pv = pp_acc.tile([P, 512], F32, tag="acc", name=f"pv{tbp}")
            wv_r = w_tiles["v"].rearrange("p (n d) -> p n d", n=NCT)
            for i in range(2):
                tb = 2 * tbp + i
                for c in range(NCT):
                    nc.tensor.matmul(
                        pv[:, i * DOUT:(i + 1) * DOUT],
                        lhsT=xsb_r[:, c, tb * P:(tb + 1) * P],
                        rhs=wv_r[:, c, :],
                        start=(c == 0),
                        stop=(c == NCT - 1),
                        skip_group_check=True,
                    )
            pv_r = pv.rearrange("p (i h d) -> p i h d", i=2, h=NHC)
            for i in range(2):
                vt_r = va[2 * tbp + i].rearrange("p (h e) -> p h e", e=HS + 1)
                nc.gpsimd.memset(vt_r[:, :, HS:HS + 1], 1.0)
                nc.vector.tensor_copy(vt_r[:, :, 0:HS], pv_r[:, i, :, :])

        # ---- projections first, q/k chunks interleaved so pair-0 rope
        # completes ASAP; V-proj tb-pairs 0,1 next (needed by first PV),
        # the rest issued as PE filler inside the pair-0 attention loop.
        for ch in range(4):
            proj_rope_chunk(0, "q", qt[0], ch)
            proj_rope_chunk(0, "k", kt[0], ch)
        for ch in range(4):
            proj_rope_chunk(1, "q", qt[1], ch)
            proj_rope_chunk(1, "k", kt[1], ch)
        vproj(0)
        vproj(1)

        def pv_head(m, hi, qc, pts):
            """PV accumulation + copy + output DMA for one head, 512-q chunk."""
            h = 2 * m + hi
            q0 = qc * 512
            jmax = 4 * qc + 3
            po = pp_po.tile([HS + 1, 512], F32, tag="po", name=f"po{h}_{qc}")
            order = [jj for jj in range(jmax + 1) if jj * P <= q0]
            order += [jj for jj in range(jmax + 1) if jj * P > q0]
            for i, jj in enumerate(order):
                col0 = max(0, jj * P - q0)
                nc.tensor.matmul(
                    po[:, col0:512],
                    lhsT=va[jj][:, h * (HS + 1):(h + 1) * (HS + 1)],
                    rhs=pts[hi][jj][:, q0 + col0 - jj * P: q0 + 512 - jj * P],
                    start=(i == 0),
                    stop=(i == jmax),
                    skip_group_check=True,
                )
            ob = opool.tile([HS + 1, 512], F32, tag="ob", name=f"ob{h}_{qc}")
            nc.vector.tensor_copy(ob[:], po[:])
            nc.sync.dma_start(
                outT[h * (HS + 1):(h + 1) * (HS + 1), q0:q0 + 512], ob[:]
            )

        # ---- attention: pairs sequential, j-major, PV lagged 2 key-blocks.
        vp_next = 2  # remaining V-proj tb-pairs used as PE filler
        for m in (0, 1):
            pts = {0: [], 1: []}
            for j in range(T // P):
                w_j = T - j * P
                ptj_pair = []
                for hi in range(2):
                    ptj = ptpool.tile(
                        [P, w_j], BF16, tag=f"pt{hi}_{j}", name=f"pt{m}_{hi}_{j}",
                        bufs=2 if j < 4 else None,
                    )
                    pts[hi].append(ptj)
                    ptj_pair.append(ptj)
                for seg0 in range(0, w_j, 1024):
                    seg = min(1024, w_j - seg0)
                    ps_pair = [
                        pp_s.tile([P, 1024], F32, tag="ps", name=f"ps{m}_{j}_{hi}")
                        for hi in range(2)
                    ]
                    for s5 in range(0, seg, 512):
                        n = min(512, seg - s5)
                        q0 = j * P + seg0 + s5
                        for hi in range(2):
                            r0 = hi * HS
                            nc.tensor.matmul(
                                ps_pair[hi][:, s5:s5 + n],
                                lhsT=kt[m][r0:r0 + HS, j * P:(j + 1) * P],
                                rhs=qt[m][r0:r0 + HS, q0:q0 + n],
                                start=True,
                                stop=True,
                                tile_position=(r0, 0),
                            )
                    for hi in range(2):
                        nc.scalar.activation(
                            ptj_pair[hi][:, seg0:seg0 + seg],
                            ps_pair[hi][:, 0:seg],
                            mybir.ActivationFunctionType.Exp,
                            scale=SCALE,
                        )
                for hi in range(2):
                    nc.vector.tensor_mul(
                        ptj_pair[hi][:, 0:P], ptj_pair[hi][:, 0:P], utri_s[:]
                    )
                # V-proj filler early in pair 0 (va 4..15 needed from qc1 on)
                if m == 0 and j < 6 and vp_next < 8:
                    vproj(vp_next)
                    vp_next += 1
                # lagged PV: qc's last score block is j=4qc+3; issue 2 later
                for qc in range(4):
                    if j == min(4 * qc + 5, T // P - 1) and (4 * qc + 3 <= j):
                        for hi in range(2):
                            pv_head(m, hi, qc, pts)

    nc.compile()
    return nc


_CACHE = {}


def _get_nc():
    if "nc" not in _CACHE:
        _CACHE["nc"] = _build_nc()
    return _CACHE["nc"]


def _host_inputs(x, Wq, Wk, Wv):
    bf = ml_dtypes.bfloat16
    f8 = ml_dtypes.float8_e4m3
    B = x.shape[0]
    # RoPE tables (match reference: theta over hs/2 freqs with dim=n_emb)
    i = np.arange(HS // 2, dtype=np.float32)
    theta = np.float32(10000.0) ** (-2.0 * i / np.float32(CIN))
    pos = np.arange(T, dtype=np.float32)
    ang = pos[:, None] * theta[None, :]
    cosT = np.cos(ang).T.astype(np.float32)  # [32, T]
    sinT = np.sin(ang).T.astype(np.float32)
    cos4 = np.ascontiguousarray(np.tile(cosT, (4, 1))).astype(bf)
    sin4 = np.ascontiguousarray(
        np.tile(np.concatenate([-sinT, sinT], axis=0), (2, 1))
    ).astype(bf)  # rows: [-sin, +sin] x2
    utri_np = np.triu(np.ones((P, P), np.float32)).astype(bf)

    perm = np.concatenate([np.arange(0, HS, 2), np.arange(1, HS, 2)])
    in_maps = []
    for core in range(8):
        b, g = core // 4, core % 4
        idx = np.concatenate([(4 * g + h) * HS + perm for h in range(NHC)])
        xT = np.ascontiguousarray(x[b].T)
        m = {
            "x8T": xT.astype(f8),
            "xbT": xT.astype(bf),
            "wq8": np.ascontiguousarray(Wq[idx].T).astype(f8),
            "wk8": np.ascontiguousarray(Wk[idx].T).astype(f8),
            "wvT": np.ascontiguousarray(Wv[g * DOUT:(g + 1) * DOUT].T).astype(bf),
            "cos4": cos4,
            "sin4": sin4,
            "utri": utri_np,
        }
        in_maps.append(m)
    return in_maps


def kernel(x, Wq, Wk, Wv, _trace=False, _trace_kwargs=None):
    x = np.asarray(x)
    Wq, Wk, Wv = np.asarray(Wq), np.asarray(Wk), np.asarray(Wv)
    B = x.shape[0]
    nc = _get_nc()
    in_maps = _host_inputs(x, Wq, Wk, Wv)
    res = run_bass_kernel_spmd(
        nc, in_maps, list(range(8)), trace=_trace, **(_trace_kwargs or {})
    )
    out = np.zeros((B, T, CIN), np.float32)
    for core in range(8):
        b, g = core // 4, core % 4
        r = res.results[core]["outT"].reshape(NHC, HS + 1, T)
        o = r[:, 0:HS, :] / r[:, HS:HS + 1, :]
        out[b, :, g * DOUT:(g + 1) * DOUT] = o.reshape(DOUT, T).T
    if _trace:
        return out, res
    return out
